# revision 1
# baseline (speedup 1.0000x reference)
"""Additive-attention pooling (nn_Meta_Module) Trainium2 kernel — v4.

Full inputs in, full output out. Pure data-parallel over 8 NeuronCores
(batch 512 -> 64/core). Per core, a Bass/Tile kernel computes
  a    = all_memory @ U.T              (PE 128x128 phases, bf16, [k,(b,s)])
  t    = tanh(a + last @ W.T)          (bias split: ActE fused bias-tanh for
                                        4/16 batches, DVE 4D-AP bias-add +
                                        big-chunk ActE tanh for 12/16)
  sc   = V.T @ t                       (PE col-tiled phases, 2 batches/MM,
                                        4-way strip concurrency)
  P    = all_memory @ MetaW.T          (PE col-tiled phases, 2 batches/MM)
  e    = exp(sc) (ActE), esum (DVE reduce)
  out  = (sum_s e * P) / esum + Metab  (PE selector bcast + DVE mult+reduce,
                                        host divide)

Matmul phases are block-contiguous to avoid PE tiling-mode-switch drains.
"""
import numpy as np
import ml_dtypes
from contextlib import ExitStack

import concourse.bass as bass
import concourse.tile as tile
import concourse.mybir as mybir
from concourse import bacc
from concourse.bass_utils import run_bass_kernel_spmd

BF16 = mybir.dt.bfloat16
F32 = mybir.dt.float32
AF = mybir.ActivationFunctionType
ALU = mybir.AluOpType
NBF = ml_dtypes.bfloat16

B, S, H = 512, 200, 256
N_CORES = 8
B_LOC = B // N_CORES      # 64 batches/core
NW = B_LOC // 2           # 32 windows of 2 batches (400 cols)
NCOL = B_LOC * S          # 12800 columns per core
NCHUNK = 8                # x DMA chunks per half
CSZ = NCOL // NCHUNK      # 1600 cols per chunk


def _ap4(base_ap, offset_elems, dims):
    """Build a 4D AP on base_ap's tensor: dims = [(stride, count), ...3 free]."""
    p = base_ap.ap[0]
    return bass.AP(tensor=base_ap.tensor, offset=base_ap.offset + offset_elems,
                   ap=[list(p)] + [list(d) for d in dims])


def _pair_bias_ap(lt_ap, col0, inner):
    """lt[:, col0:col0+2] broadcast along s -> [128, 2, inner] AP."""
    return lt_ap[:, col0:col0 + 2, None].to_broadcast((128, 2, inner))


def _split_cols(ap2d, outer, inner):
    """[128, outer*inner] AP -> [128, outer, inner] AP (row-major split)."""
    p = ap2d.ap[0]
    return bass.AP(tensor=ap2d.tensor, offset=ap2d.offset,
                   ap=[list(p), [inner, outer], [1, inner]])


def build_nc(b_loc=B_LOC, debug=False):
    assert b_loc == 64
    nc = bacc.Bacc("TRN2", target_bir_lowering=False, debug=debug)

    allT = [nc.dram_tensor(f"allT{h}", [128, NCOL], BF16, kind="ExternalInput")
            for h in range(2)]
    CB_d = nc.dram_tensor("CB", [128, 1152], BF16, kind="ExternalInput")
    LT_d = nc.dram_tensor("LT", [128, 2 * b_loc], F32, kind="ExternalInput")
    numer_d = nc.dram_tensor("numer", [128, 4], F32, kind="ExternalOutput")
    esum_d = nc.dram_tensor("esum", [128, 4], F32, kind="ExternalOutput")

    with tile.TileContext(nc) as tc, ExitStack() as ctx:
        big = ctx.enter_context(tc.tile_pool(name="big", bufs=1))
        misc = ctx.enter_context(tc.tile_pool(name="misc", bufs=1))
        apool = ctx.enter_context(tc.tile_pool(name="apool", bufs=4, space="PSUM"))
        vpool = ctx.enter_context(tc.tile_pool(name="vpool", bufs=2, space="PSUM"))
        ptpool = ctx.enter_context(tc.tile_pool(name="ptpool", bufs=2, space="PSUM"))

        cb = big.tile([128, 1152], BF16, tag="cb")
        nc.scalar.dma_start(cb[:], CB_d.ap())
        lt = big.tile([128, 2 * b_loc], F32, tag="lt")
        nc.scalar.dma_start(lt[:], LT_d.ap())

        ut = cb[:, 0:512]
        vsp = cb[:, 512:768]
        mwp = cb[:, 768:1024]
        sel = cb[:, 1024:1152]

        def UT(h, k):
            return ut[:, (2 * h + k) * 128:(2 * h + k + 1) * 128]

        def VSP(k, c):
            return vsp[:, (k * 4 + c) * 32:(k * 4 + c) * 32 + 32]

        def MW(h, i8):
            return mwp[:, (h * 4 + i8) * 32:(h * 4 + i8) * 32 + 32]

        x = [big.tile([128, NCOL], BF16, tag=f"x{h}", name=f"x{h}")
             for h in range(2)]

        # scalar's HWDGE queue sustains ~220 GB/s, sync's only ~60 (head-of-
        # line blocked by framework semaphores): scalar carries h0 fully and
        # early h1; sync only the late h1 chunks. All issued up front.
        for _c in range(NCHUNK):
            nc.scalar.dma_start(x[0][:, _c * CSZ:(_c + 1) * CSZ],
                                allT[0].ap()[:, _c * CSZ:(_c + 1) * CSZ])
            eng = nc.scalar if _c < 4 else nc.sync
            eng.dma_start(x[1][:, _c * CSZ:(_c + 1) * CSZ],
                          allT[1].ap()[:, _c * CSZ:(_c + 1) * CSZ])

        # warm the exp/tanh activation table while the x DMA streams
        dummy = misc.tile([128, 1], BF16, tag="dummy")
        nc.scalar.activation(dummy[:], lt[:, 0:1], AF.Tanh)


        arg = big.tile([128, 2 * NCOL], BF16, tag="arg")
        tts = [big.tile([128, NCOL], BF16, tag=f"tts{h}", name=f"tts{h}")
               for h in range(2)]

        e_full = [misc.tile([128, 2 * S], BF16, tag=f"ef{g}", name=f"ef{g}")
                  for g in range(2)]
        esum_sb = misc.tile([128, 4], F32, tag="esum")
        numer = misc.tile([128, 4], F32, tag="numer")
        pt_sb = [misc.tile([128, 2 * S], F32, tag=f"ptsb{g}", name=f"ptsb{g}")
                 for g in range(2)]
        prod = [misc.tile([128, 2 * S], F32, tag=f"prod{g}", name=f"prod{g}")
                for g in range(2)]

        vps = [None, None]
        PT = [None, None]

        def emit_V(win):
            """Scores for the 2 batches of window win -> vps[win//16]."""
            g2, wl = divmod(win, 16)
            if vps[g2] is None:
                vps[g2] = vpool.tile([128, 512], F32, tag="vps", name=f"vps{g2}")
            j, c = wl % 4, wl // 4
            first = (wl < 4)
            for k in range(2):
                nc.tensor.matmul(
                    vps[g2][32 * j:32 * j + 32, 0:2 * S], VSP(k, c),
                    tts[k][:, 400 * win:400 * (win + 1)],
                    start=(first and k == 0), stop=(k == 1),
                    tile_position=(0, 32 * j), skip_group_check=True)

        def emit_P(pg):
            """MetaW projection for the 2 batches of window pg -> PT[pg//16]."""
            g2, pl = divmod(pg, 16)
            if PT[g2] is None:
                PT[g2] = ptpool.tile([128, 512], F32, tag="pt", name=f"pt{g2}")
            j, i8 = (pl + 2) % 4, pl // 4
            first = (pl < 4)
            for h in range(2):
                nc.tensor.matmul(
                    PT[g2][32 * j:32 * j + 32, 0:2 * S], MW(h, i8),
                    x[h][:, 400 * pg:400 * (pg + 1)],
                    start=(first and h == 0), stop=(h == 1),
                    tile_position=(0, 32 * j), skip_group_check=True)

        def endgame_front(g2):
            """exp + esum + PT copy (ActE/DVE parts, no PE)."""
            nc.scalar.activation(e_full[g2][:], vps[g2][:, 0:2 * S], AF.Exp)
            nc.vector.tensor_reduce(
                esum_sb[:, 2 * g2:2 * g2 + 2], _split_cols(e_full[g2][:], 2, S),
                axis=mybir.AxisListType.X, op=ALU.add)
            nc.vector.tensor_copy(pt_sb[g2][:], PT[g2][:, 0:2 * S])

        def endgame_back(g2):
            """erep matmul (PE, 128x128 mode) + weighted reduce (DVE)."""
            erep = vpool.tile([128, 512], F32, tag="vps", name=f"erep{g2}")
            nc.tensor.matmul(erep[:, 0:2 * S], sel, e_full[g2][:],
                             start=True, stop=True)
            nc.vector.tensor_mul(prod[g2][:], pt_sb[g2][:], erep[:, 0:2 * S])
            nc.vector.tensor_reduce(
                numer[:, 2 * g2:2 * g2 + 2], _split_cols(prod[g2][:], 2, S),
                axis=mybir.AxisListType.X, op=ALU.add)

        next_chunk = [2]

        def vp_burst(H_v, H_p):
            """Col-tiled burst: V for half-block H_v, P for half-block H_p."""
            for i in range(4):
                if H_v >= 0:
                    emit_V(4 * H_v + i)
                if 0 <= H_p <= 7:
                    emit_P(4 * H_p + i)

        for q in range(4):                  # blocks of 8 windows
            if q == 3:
                endgame_back(0)             # erep joins the 128x128 region
            for r8 in range(8):
                w = 8 * q + r8
                b0 = 2 * w
                H = 2 * q + r8 // 4         # half-block index 0..7
                at = {k: apool.tile([128, 512], F32, tag="a", name=f"a_{w}_{k}")
                      for k in range(2)}
                for k in range(2):
                    for h in range(2):
                        nc.tensor.matmul(
                            at[k][:, 0:400], UT(h, k),
                            x[h][:, 400 * w:400 * (w + 1)],
                            start=(h == 0), stop=(h == 1))
                    # consumer for this k-half starts while k1 matmuls run
                    if r8 % 4 != 3:         # DVE bias path
                        in0 = _split_cols(at[k][:, 0:2 * S], 2, S)
                        out_ap = _split_cols(
                            arg[:, k * NCOL + 400 * w:k * NCOL + 400 * (w + 1)],
                            2, S)
                        in1 = _pair_bias_ap(lt, k * b_loc + b0, S)
                        nc.vector.tensor_tensor(out_ap, in0, in1, ALU.add)
                    else:                   # ActE fused bias-tanh path
                        for i in range(2):
                            b = b0 + i
                            nc.scalar.activation(
                                tts[k][:, S * b:S * (b + 1)],
                                at[k][:, S * i:S * (i + 1)],
                                AF.Tanh,
                                bias=lt[:, k * b_loc + b:k * b_loc + b + 1])
                if r8 in (2, 6):            # big tanh piece for 6 DVE batches
                    lo = 3200 * q + 1600 * (r8 // 4)
                    for k in range(2):
                        nc.scalar.activation(
                            tts[k][:, lo:lo + 1200],
                            arg[:, k * NCOL + lo:k * NCOL + lo + 1200], AF.Tanh)
                if r8 in (3, 7):            # col-tiled VP burst per half-block
                    vp_burst(H - 2, H - 1)
                    if q == 2 and r8 == 7:
                        endgame_front(0)
        vp_burst(6, 7)
        vp_burst(7, -2)
        endgame_front(1)
        endgame_back(1)
        nc.sync.dma_start(numer_d.ap(), numer[:])
        nc.sync.dma_start(esum_d.ap(), esum_sb[:])
    nc.compile()
    return nc


def prep_core_inputs(all_c, last_c, U, W, V, MetaW, b_loc=B_LOC):
    x = np.ascontiguousarray(all_c.transpose(2, 0, 1)).astype(NBF)  # [H, b, S]
    m = {}
    m["allT0"] = np.ascontiguousarray(x[:128].reshape(128, b_loc * S))
    m["allT1"] = np.ascontiguousarray(x[128:].reshape(128, b_loc * S))
    l = (last_c @ W.T).astype(np.float32)
    m["LT"] = np.ascontiguousarray(
        l.T.reshape(2, 128, b_loc).transpose(1, 0, 2).reshape(128, 2 * b_loc))
    ut = U.reshape(2, 128, 2, 128).transpose(3, 2, 0, 1).reshape(128, 512)
    vsp = np.zeros((128, 256), np.float32)
    for k in range(2):
        for c in range(4):
            vsp[:, (k * 4 + c) * 32 + c] = V[128 * k:128 * (k + 1), 0]
    mwp = np.zeros((128, 2, 4, 32), np.float32)
    for h in range(2):
        for i in range(4):
            mwp[:, h, i, 4 * i:4 * i + 4] = MetaW[:, 128 * h:128 * (h + 1)].T
    mwp = mwp.reshape(128, 256)
    sel = np.zeros((128, 128), np.float32)
    for wl in range(16):
        p = 32 * (wl % 4) + wl // 4
        base = 32 * ((wl + 2) % 4) + 4 * (wl // 4)
        sel[p, base:base + 4] = 1.0
    m["CB"] = np.ascontiguousarray(
        np.concatenate([ut, vsp, mwp, sel], axis=1)).astype(NBF)
    return m


def postprocess_core(numer, esum, Metab, b_loc=B_LOC):
    out = np.empty((b_loc, 4), np.float32)
    for b in range(b_loc):
        g2, bg = divmod(b, 32)
        wl, par = divmod(bg, 2)
        prow = 32 * ((wl + 2) % 4) + 4 * (wl // 4)
        vrow = 32 * (wl % 4) + wl // 4
        out[b] = numer[prow:prow + 4, 2 * g2 + par] / esum[vrow, 2 * g2 + par]
    return out + Metab.reshape(1, 4)


_cache = {}


def _get_nc():
    if "nc" not in _cache:
        _cache["nc"] = build_nc(B_LOC)
    return _cache["nc"]


def kernel(all_memory, last_memory, U, W, V, MetaW, Metab):
    all_memory = np.asarray(all_memory, dtype=np.float32)
    last_memory = np.asarray(last_memory, dtype=np.float32)
    U = np.asarray(U, dtype=np.float32)
    W = np.asarray(W, dtype=np.float32)
    V = np.asarray(V, dtype=np.float32)
    MetaW = np.asarray(MetaW, dtype=np.float32)
    Metab = np.asarray(Metab, dtype=np.float32)
    nc = _get_nc()
    in_maps = []
    for c in range(N_CORES):
        sl = slice(c * B_LOC, (c + 1) * B_LOC)
        in_maps.append(prep_core_inputs(
            all_memory[sl], last_memory[sl], U, W, V, MetaW))
    res = run_bass_kernel_spmd(nc, in_maps, core_ids=list(range(N_CORES)))
    outs = [postprocess_core(res.results[c]["numer"], res.results[c]["esum"],
                             Metab) for c in range(N_CORES)]
    return np.concatenate(outs, axis=0).astype(np.float32)



# revision 3
# speedup vs baseline: 1.1117x; 1.1117x over previous
"""Additive-attention pooling (nn_Meta_Module) Trainium2 kernel — v5.

Full inputs in, full output out. Pure data-parallel over 8 NeuronCores
(batch 512 -> 64/core). Per core, a Bass/Tile kernel computes
  a    = all_memory @ U.T            (PE 128x128, bf16, [k,(b,s)] layout)
  t    = tanh(a + last @ W.T)        (DVE one-shot bias drain per window
                                      [FD=800, both k-halves] + ActE
                                      big-chunk tanh; 4 windows use the
                                      ActE fused bias-tanh path instead)
  sc   = V.T @ t                     (PE col-tiled strips, 2 batches/MM)
  P    = all_memory @ MetaW.T        (PE col-tiled strips, same PSUM bank
                                      as sc -- disjoint partition rows)
  out: scores+projections shipped to host; softmax + the tiny O(B*S*4)
  e-weighted combine + bias run on host in f64/f32.

Startup: PE warmed with dummy matmuls during the DMA fill so real MMs run
at 2.4 GHz; x streamed on sync+gpsimd DMA rings so the scalar (ActE) ring
never blocks activations.
"""
import numpy as np
import ml_dtypes
from contextlib import ExitStack

import concourse.bass as bass
import concourse.tile as tile
import concourse.mybir as mybir
from concourse import bacc
from concourse.bass_utils import run_bass_kernel_spmd

BF16 = mybir.dt.bfloat16
F32 = mybir.dt.float32
AF = mybir.ActivationFunctionType
ALU = mybir.AluOpType
NBF = ml_dtypes.bfloat16

B, S, H = 512, 200, 256
N_CORES = 8
B_LOC = B // N_CORES      # 64 batches/core
NW = B_LOC // 2           # 32 windows of 2 batches (400 cols)
NCOL = B_LOC * S          # 12800 columns per core

# windows whose bias+tanh run fused on ActE straight from PSUM
FUSED = (3, 11, 27, 30)
# contiguous DVE-drained runs -> one big ActE tanh chunk per k-half,
# emitted right after the last drain of the run
CHUNK_AT = {2: (0, 2), 7: (4, 7), 10: (8, 10), 15: (12, 15), 19: (16, 19),
            23: (20, 23), 26: (24, 26), 29: (28, 29), 31: (31, 31)}
# V/P strip bursts issued after window w's 'a' matmuls
BURSTS = {3: (("P", 0),), 7: (("V", 0), ("P", 1)), 11: (("V", 1), ("P", 2)),
          15: (("V", 2), ("P", 3)), 19: (("V", 3), ("P", 4)),
          23: (("V", 4), ("P", 5)), 27: (("V", 5), ("P", 6)),
          29: (("V", 6),), 31: (("P", 7), ("V", 7))}
# x DMA chunks (col offset, cols): sized so early windows unblock fast
CHUNKS = ((0, 400), (400, 800), (1200, 1600), (2800, 2400), (5200, 3200),
          (8400, 4400))


def _ap(base_ap, offset_elems, dims):
    """AP on base_ap's tensor: dims = [(stride, count), ...] free dims."""
    p = base_ap.ap[0]
    return bass.AP(tensor=base_ap.tensor, offset=base_ap.offset + offset_elems,
                   ap=[list(p)] + [list(d) for d in dims])


def build_nc(debug=False):
    nc = bacc.Bacc("TRN2", target_bir_lowering=False, debug=debug)

    allT = [nc.dram_tensor(f"allT{h}", [128, NCOL], BF16, kind="ExternalInput")
            for h in range(2)]
    CB_d = nc.dram_tensor("CB", [128, 1024], BF16, kind="ExternalInput")
    LT_d = nc.dram_tensor("LT", [128, 128], F32, kind="ExternalInput")
    SCPT_d = nc.dram_tensor("SCPT", [128, 800], F32, kind="ExternalOutput")

    with tile.TileContext(nc) as tc, ExitStack() as ctx:
        big = ctx.enter_context(tc.tile_pool(name="big", bufs=1))
        misc = ctx.enter_context(tc.tile_pool(name="misc", bufs=1))
        apool = ctx.enter_context(tc.tile_pool(name="apool", bufs=3, space="PSUM"))
        vppool = ctx.enter_context(tc.tile_pool(name="vppool", bufs=2, space="PSUM"))

        scratch = misc.tile([128, 384], BF16, tag="scratch")
        nc.gpsimd.memset(scratch[:], 0.125)

        cb = big.tile([128, 1024], BF16, tag="cb")
        lt = big.tile([128, 128], F32, tag="lt")
        x = [big.tile([128, NCOL], BF16, tag=f"x{h}", name=f"x{h}")
             for h in range(2)]

        # DMA: sync ring carries CB+LT+x0; gpsimd (SWDGE) ring carries x1.
        # The scalar ring stays empty so ActE never queues behind DIRECT2Ds.
        nc.sync.dma_start(cb[:], CB_d.ap())
        nc.sync.dma_start(lt[:], LT_d.ap())
        for off, size in CHUNKS:
            nc.sync.dma_start(x[0][:, off:off + size],
                              allT[0].ap()[:, off:off + size])
        for off, size in CHUNKS:
            nc.gpsimd.dma_start(x[1][:, off:off + size],
                                allT[1].ap()[:, off:off + size])

        # warm the tanh activation table while DMA streams
        dummy = misc.tile([128, 1], BF16, tag="dummy")
        nc.scalar.activation(dummy[:], scratch[:, 0:1], AF.Tanh)

        ut = cb[:, 0:512]
        vsp = cb[:, 512:768]
        mwp = cb[:, 768:1024]

        def UT(h, k):
            return ut[:, (2 * h + k) * 128:(2 * h + k + 1) * 128]

        def VSP(k, c):
            return vsp[:, (k * 4 + c) * 32:(k * 4 + c) * 32 + 32]

        def MW(h, i8):
            return mwp[:, (h * 4 + i8) * 32:(h * 4 + i8) * 32 + 32]

        arg = big.tile([128, 2 * NCOL], BF16, tag="arg")
        tts = [big.tile([128, NCOL], BF16, tag=f"tts{h}", name=f"tts{h}")
               for h in range(2)]
        scpt_sb = misc.tile([128, 800], F32, tag="scpt")

        # PE warmup: ~4us of dummy matmuls so HAM unthrottles to 2.4 GHz
        # before the real stream starts. Results land in the first apool
        # slot and are overwritten by window 5's start=True matmuls.
        warm = apool.tile([128, 1024], F32, tag="a", name="warm")
        for _ in range(12):
            nc.tensor.matmul(warm[:, 0:384], scratch[:, 0:128],
                             scratch[:, 0:384], start=True, stop=True)

        vp = [None, None]
        first_touch = {}

        def emit_V(win):
            g2, wl = divmod(win, 16)
            if vp[g2] is None:
                vp[g2] = vppool.tile([128, 512], F32, tag="vp", name=f"vp{g2}")
            j, c = wl % 4, wl // 4
            ft = first_touch.setdefault((g2, j), [True])
            for k in range(2):
                nc.tensor.matmul(
                    vp[g2][32 * j:32 * j + 32, 0:2 * S], VSP(k, c),
                    tts[k][:, 400 * win:400 * (win + 1)],
                    start=(ft[0] and k == 0), stop=(k == 1),
                    tile_position=(0, 32 * j), skip_group_check=True)
            ft[0] = False

        def emit_P(pg):
            g2, pl = divmod(pg, 16)
            if vp[g2] is None:
                vp[g2] = vppool.tile([128, 512], F32, tag="vp", name=f"vp{g2}")
            j, i8 = (pl + 2) % 4, pl // 4
            ft = first_touch.setdefault((g2, j), [True])
            for h in range(2):
                nc.tensor.matmul(
                    vp[g2][32 * j:32 * j + 32, 0:2 * S], MW(h, i8),
                    x[h][:, 400 * pg:400 * (pg + 1)],
                    start=(ft[0] and h == 0), stop=(h == 1),
                    tile_position=(0, 32 * j), skip_group_check=True)
            ft[0] = False

        def ship(g2):
            nc.vector.tensor_copy(scpt_sb[:, 400 * g2:400 * (g2 + 1)],
                                  vp[g2][:, 0:2 * S])
            nc.sync.dma_start(SCPT_d.ap()[:, 400 * g2:400 * (g2 + 1)],
                              scpt_sb[:, 400 * g2:400 * (g2 + 1)])

        for w in range(NW):
            A = apool.tile([128, 1024], F32, tag="a", name=f"a{w}")
            for k in range(2):
                for h in range(2):
                    nc.tensor.matmul(
                        A[:, 512 * k:512 * k + 400], UT(h, k),
                        x[h][:, 400 * w:400 * (w + 1)],
                        start=(h == 0), stop=(h == 1))
            if w in FUSED:          # ActE fused bias+tanh straight from PSUM
                for k in range(2):
                    for i in range(2):
                        b = 2 * w + i
                        nc.scalar.activation(
                            tts[k][:, S * b:S * (b + 1)],
                            A[:, 512 * k + S * i:512 * k + S * (i + 1)],
                            AF.Tanh, bias=lt[:, 64 * k + b:64 * k + b + 1])
            else:                   # one DVE op drains both k-halves + bias
                in0 = _ap(A[:, :], 0, [(512, 2), (200, 2), (1, 200)])
                in1 = _ap(lt[:, :], 2 * w, [(64, 2), (1, 2), (0, 200)])
                out = _ap(arg[:, :], 400 * w, [(NCOL, 2), (200, 2), (1, 200)])
                nc.vector.tensor_tensor(out, in0, in1, ALU.add)
            if w in CHUNK_AT:       # big ActE tanh over the finished run
                w0, w1 = CHUNK_AT[w]
                for k in range(2):
                    nc.scalar.activation(
                        tts[k][:, 400 * w0:400 * (w1 + 1)],
                        arg[:, k * NCOL + 400 * w0:k * NCOL + 400 * (w1 + 1)],
                        AF.Tanh)
            for kind, hh in BURSTS.get(w, ()):
                for i in range(4):
                    (emit_V if kind == "V" else emit_P)(4 * hh + i)
            if w == 20:
                ship(0)
        ship(1)
    nc.compile()
    return nc


def prep_core_inputs(all_c, last_c, U, W, V, MetaW, b_loc=B_LOC):
    x = np.ascontiguousarray(all_c.transpose(2, 0, 1)).astype(NBF)  # [H, b, S]
    m = {}
    m["allT0"] = np.ascontiguousarray(x[:128].reshape(128, b_loc * S))
    m["allT1"] = np.ascontiguousarray(x[128:].reshape(128, b_loc * S))
    l = (last_c @ W.T).astype(np.float32)
    m["LT"] = np.ascontiguousarray(
        l.T.reshape(2, 128, b_loc).transpose(1, 0, 2).reshape(128, 2 * b_loc))
    ut = U.reshape(2, 128, 2, 128).transpose(3, 2, 0, 1).reshape(128, 512)
    vsp = np.zeros((128, 256), np.float32)
    for k in range(2):
        for c in range(4):
            vsp[:, (k * 4 + c) * 32 + 16 + c] = V[128 * k:128 * (k + 1), 0]
    mwp = np.zeros((128, 2, 4, 32), np.float32)
    for h in range(2):
        for i in range(4):
            mwp[:, h, i, 4 * i:4 * i + 4] = MetaW[:, 128 * h:128 * (h + 1)].T
    mwp = mwp.reshape(128, 256)
    m["CB"] = np.ascontiguousarray(
        np.concatenate([ut, vsp, mwp], axis=1)).astype(NBF)
    return m


def postprocess_core(scpt, Metab, b_loc=B_LOC):
    """scpt [128, 800] f32: per g2-half, V rows hold scores, P rows hold the
    MetaW projections. Softmax + e-weighted combine on host."""
    sc = np.empty((b_loc, S), np.float32)
    P = np.empty((b_loc, 4, S), np.float32)
    for g2 in range(2):
        blk = scpt[:, 400 * g2:400 * (g2 + 1)]
        for wl in range(16):
            vrow = 32 * (wl % 4) + 16 + wl // 4
            prow = 32 * ((wl + 2) % 4) + 4 * (wl // 4)
            for par in range(2):
                b = 32 * g2 + 2 * wl + par
                sc[b] = blk[vrow, 200 * par:200 * (par + 1)]
                P[b] = blk[prow:prow + 4, 200 * par:200 * (par + 1)]
    sc64 = sc.astype(np.float64)
    e = np.exp(sc64 - sc64.max(axis=1, keepdims=True))
    alpha = (e / e.sum(axis=1, keepdims=True)).astype(np.float32)
    return np.einsum('bs,bms->bm', alpha, P) + Metab.reshape(1, 4)


_cache = {}


def _get_nc():
    if "nc" not in _cache:
        _cache["nc"] = build_nc()
    return _cache["nc"]


def kernel(all_memory, last_memory, U, W, V, MetaW, Metab):
    all_memory = np.asarray(all_memory, dtype=np.float32)
    last_memory = np.asarray(last_memory, dtype=np.float32)
    U = np.asarray(U, dtype=np.float32)
    W = np.asarray(W, dtype=np.float32)
    V = np.asarray(V, dtype=np.float32)
    MetaW = np.asarray(MetaW, dtype=np.float32)
    Metab = np.asarray(Metab, dtype=np.float32)
    nc = _get_nc()
    in_maps = []
    for c in range(N_CORES):
        sl = slice(c * B_LOC, (c + 1) * B_LOC)
        in_maps.append(prep_core_inputs(
            all_memory[sl], last_memory[sl], U, W, V, MetaW))
    res = run_bass_kernel_spmd(nc, in_maps, core_ids=list(range(N_CORES)))
    outs = [postprocess_core(res.results[c]["SCPT"], Metab)
            for c in range(N_CORES)]
    return np.concatenate(outs, axis=0).astype(np.float32)


# revision 9
# speedup vs baseline: 1.1236x; 1.0107x over previous
"""Additive-attention pooling (nn_Meta_Module) Trainium2 kernel — v5.

Full inputs in, full output out. Pure data-parallel over 8 NeuronCores
(batch 512 -> 64/core). Per core, a Bass/Tile kernel computes
  a    = all_memory @ U.T            (PE 128x128, bf16, [k,(b,s)] layout)
  t    = tanh(a + last @ W.T)        (DVE one-shot bias drain per window
                                      [FD=800, both k-halves] + ActE
                                      big-chunk tanh; 4 windows use the
                                      ActE fused bias-tanh path instead)
  sc   = V.T @ t                     (PE col-tiled strips, 2 batches/MM)
  P    = all_memory @ MetaW.T        (PE col-tiled strips, same PSUM bank
                                      as sc -- disjoint partition rows)
  out: scores+projections shipped to host; softmax + the tiny O(B*S*4)
  e-weighted combine + bias run on host in f64/f32.

Startup: PE warmed with dummy matmuls during the DMA fill so real MMs run
at 2.4 GHz; x streamed on sync+gpsimd DMA rings so the scalar (ActE) ring
never blocks activations.
"""
import numpy as np
import ml_dtypes
from contextlib import ExitStack

import concourse.bass as bass
import concourse.tile as tile
import concourse.mybir as mybir
from concourse import bacc
from concourse.bass_utils import run_bass_kernel_spmd

BF16 = mybir.dt.bfloat16
F32 = mybir.dt.float32
AF = mybir.ActivationFunctionType
ALU = mybir.AluOpType
NBF = ml_dtypes.bfloat16

B, S, H = 512, 200, 256
N_CORES = 8
B_LOC = B // N_CORES      # 64 batches/core
NW = B_LOC // 2           # 32 windows of 2 batches (400 cols)
NCOL = B_LOC * S          # 12800 columns per core

# windows whose bias+tanh run fused on ActE straight from PSUM
FUSED = (3, 11, 27, 30)
# contiguous DVE-drained runs -> one big ActE tanh chunk (both k-halves in
# a single instruction), emitted right after the last drain of the run
CHUNK_AT = {2: (0, 2), 7: (4, 7), 10: (8, 10), 15: (12, 15), 19: (16, 19),
            23: (20, 23), 26: (24, 26), 29: (28, 29), 31: (31, 31)}
# V/P strip bursts issued after window w's 'a' matmuls; V lags 8 windows so
# the tanh chunk feeding it is always done (no PE stall on ActE)
BURSTS = {3: (("P", 0),), 7: (("P", 1),), 8: (("V", 0),),
          11: (("P", 2),), 12: (("V", 1),), 15: (("P", 3),),
          16: (("V", 2),), 19: (("P", 4),), 20: (("V", 3),),
          23: (("P", 5),), 24: (("V", 4),), 27: (("P", 6),),
          28: (("V", 5),), 30: (("V", 6),), 31: (("P", 7), ("V", 7))}
# x DMA chunks (col offset, cols): sized so early windows unblock fast
CHUNKS = ((0, 400), (400, 800), (1200, 1600), (2800, 2400), (5200, 3200),
          (8400, 4400))


def _ap(base_ap, offset_elems, dims):
    """AP on base_ap's tensor: dims = [(stride, count), ...] free dims."""
    p = base_ap.ap[0]
    return bass.AP(tensor=base_ap.tensor, offset=base_ap.offset + offset_elems,
                   ap=[list(p)] + [list(d) for d in dims])


def build_nc(debug=False):
    nc = bacc.Bacc("TRN2", target_bir_lowering=False, debug=debug)

    allT = [nc.dram_tensor(f"allT{h}", [128, NCOL], BF16, kind="ExternalInput")
            for h in range(2)]
    CB_d = nc.dram_tensor("CB", [128, 1024], BF16, kind="ExternalInput")
    LT_d = nc.dram_tensor("LT", [128, 128], F32, kind="ExternalInput")
    SCPT_d = nc.dram_tensor("SCPT", [128, 800], F32, kind="ExternalOutput")

    with tile.TileContext(nc) as tc, ExitStack() as ctx:
        big = ctx.enter_context(tc.tile_pool(name="big", bufs=1))
        misc = ctx.enter_context(tc.tile_pool(name="misc", bufs=1))
        apool = ctx.enter_context(tc.tile_pool(name="apool", bufs=3, space="PSUM"))
        vppool = ctx.enter_context(tc.tile_pool(name="vppool", bufs=2, space="PSUM"))

        scratch = misc.tile([128, 384], BF16, tag="scratch")
        nc.gpsimd.memset(scratch[:], 0.125)

        cb = big.tile([128, 1024], BF16, tag="cb")
        lt = big.tile([128, 128], F32, tag="lt")
        x = [big.tile([128, NCOL], BF16, tag=f"x{h}", name=f"x{h}")
             for h in range(2)]

        # DMA: sync ring carries x0c0+CB+LT+x0 rest; gpsimd (SWDGE) ring
        # carries x1. The scalar ring stays empty so ActE never queues
        # behind DIRECT2Ds.
        off0, size0 = CHUNKS[0]
        nc.sync.dma_start(x[0][:, off0:off0 + size0],
                          allT[0].ap()[:, off0:off0 + size0])
        nc.sync.dma_start(cb[:], CB_d.ap())
        nc.sync.dma_start(lt[:], LT_d.ap())
        for off, size in CHUNKS[1:]:
            nc.sync.dma_start(x[0][:, off:off + size],
                              allT[0].ap()[:, off:off + size])
        for off, size in CHUNKS:
            nc.gpsimd.dma_start(x[1][:, off:off + size],
                                allT[1].ap()[:, off:off + size])

        # warm the tanh activation table while DMA streams
        dummy = misc.tile([128, 1], BF16, tag="dummy")
        nc.scalar.activation(dummy[:], scratch[:, 0:1], AF.Tanh)

        ut = cb[:, 0:512]
        vsp = cb[:, 512:768]
        mwp = cb[:, 768:1024]

        def UT(h, k):
            return ut[:, (2 * h + k) * 128:(2 * h + k + 1) * 128]

        def VSP(k, c):
            return vsp[:, (k * 4 + c) * 32:(k * 4 + c) * 32 + 32]

        def MW(h, i8):
            return mwp[:, (h * 4 + i8) * 32:(h * 4 + i8) * 32 + 32]

        arg = big.tile([128, 2 * NCOL], BF16, tag="arg")
        tts = big.tile([128, 2 * NCOL], BF16, tag="tts")
        scpt_sb = misc.tile([128, 800], F32, tag="scpt")

        # PE warmup: ~4us of dummy matmuls so HAM unthrottles to 2.4 GHz
        # before the real stream starts. Results land in the first apool
        # slot and are overwritten by window 5's start=True matmuls.
        warm = apool.tile([128, 1024], F32, tag="a", name="warm")
        for _ in range(12):
            nc.tensor.matmul(warm[:, 0:384], scratch[:, 0:128],
                             scratch[:, 0:384], start=True, stop=True)

        vp = [None, None]
        first_touch = {}

        def emit_V(win):
            g2, wl = divmod(win, 16)
            if vp[g2] is None:
                vp[g2] = vppool.tile([128, 512], F32, tag="vp", name=f"vp{g2}")
            j, c = wl % 4, wl // 4
            ft = first_touch.setdefault((g2, j), [True])
            for k in range(2):
                nc.tensor.matmul(
                    vp[g2][32 * j:32 * j + 32, 0:2 * S], VSP(k, c),
                    tts[:, k * NCOL + 400 * win:k * NCOL + 400 * (win + 1)],
                    start=(ft[0] and k == 0), stop=(k == 1),
                    tile_position=(0, 32 * j), skip_group_check=True)
            ft[0] = False

        def emit_P(pg):
            g2, pl = divmod(pg, 16)
            if vp[g2] is None:
                vp[g2] = vppool.tile([128, 512], F32, tag="vp", name=f"vp{g2}")
            j, i8 = (pl + 2) % 4, pl // 4
            ft = first_touch.setdefault((g2, j), [True])
            for h in range(2):
                nc.tensor.matmul(
                    vp[g2][32 * j:32 * j + 32, 0:2 * S], MW(h, i8),
                    x[h][:, 400 * pg:400 * (pg + 1)],
                    start=(ft[0] and h == 0), stop=(h == 1),
                    tile_position=(0, 32 * j), skip_group_check=True)
            ft[0] = False

        def ship(g2):
            nc.vector.tensor_copy(scpt_sb[:, 400 * g2:400 * (g2 + 1)],
                                  vp[g2][:, 0:2 * S])
            nc.sync.dma_start(SCPT_d.ap()[:, 400 * g2:400 * (g2 + 1)],
                              scpt_sb[:, 400 * g2:400 * (g2 + 1)])

        for w in range(NW):
            A = apool.tile([128, 1024], F32, tag="a", name=f"a{w}")
            for k in range(2):
                for h in range(2):
                    nc.tensor.matmul(
                        A[:, 512 * k:512 * k + 400], UT(h, k),
                        x[h][:, 400 * w:400 * (w + 1)],
                        start=(h == 0), stop=(h == 1))
            if w in FUSED:          # ActE fused bias+tanh straight from PSUM
                for k in range(2):
                    for i in range(2):
                        b = 2 * w + i
                        nc.scalar.activation(
                            tts[:, k * NCOL + S * b:k * NCOL + S * (b + 1)],
                            A[:, 512 * k + S * i:512 * k + S * (i + 1)],
                            AF.Tanh, bias=lt[:, 64 * k + b:64 * k + b + 1])
            else:                   # one DVE op drains both k-halves + bias
                in0 = _ap(A[:, :], 0, [(512, 2), (200, 2), (1, 200)])
                in1 = _ap(lt[:, :], 2 * w, [(64, 2), (1, 2), (0, 200)])
                out = _ap(arg[:, :], 400 * w, [(NCOL, 2), (200, 2), (1, 200)])
                nc.vector.tensor_tensor(out, in0, in1, ALU.add)
            if w in CHUNK_AT:       # big ActE tanh over the finished run,
                w0, w1 = CHUNK_AT[w]    # both k-halves in one instruction
                cols = 400 * (w1 + 1 - w0)
                nc.scalar.activation(
                    _ap(tts[:, :], 400 * w0, [(NCOL, 2), (1, cols)]),
                    _ap(arg[:, :], 400 * w0, [(NCOL, 2), (1, cols)]),
                    AF.Tanh)
            for kind, hh in BURSTS.get(w, ()):
                for i in range(4):
                    (emit_V if kind == "V" else emit_P)(4 * hh + i)
            if w == 21:
                ship(0)
        ship(1)
    nc.compile()
    return nc


def prep_core_inputs(all_c, last_c, U, W, V, MetaW, b_loc=B_LOC):
    x = np.ascontiguousarray(all_c.transpose(2, 0, 1)).astype(NBF)  # [H, b, S]
    m = {}
    m["allT0"] = np.ascontiguousarray(x[:128].reshape(128, b_loc * S))
    m["allT1"] = np.ascontiguousarray(x[128:].reshape(128, b_loc * S))
    l = (last_c @ W.T).astype(np.float32)
    m["LT"] = np.ascontiguousarray(
        l.T.reshape(2, 128, b_loc).transpose(1, 0, 2).reshape(128, 2 * b_loc))
    ut = U.reshape(2, 128, 2, 128).transpose(3, 2, 0, 1).reshape(128, 512)
    vsp = np.zeros((128, 256), np.float32)
    for k in range(2):
        for c in range(4):
            vsp[:, (k * 4 + c) * 32 + 16 + c] = V[128 * k:128 * (k + 1), 0]
    mwp = np.zeros((128, 2, 4, 32), np.float32)
    for h in range(2):
        for i in range(4):
            mwp[:, h, i, 4 * i:4 * i + 4] = MetaW[:, 128 * h:128 * (h + 1)].T
    mwp = mwp.reshape(128, 256)
    m["CB"] = np.ascontiguousarray(
        np.concatenate([ut, vsp, mwp], axis=1)).astype(NBF)
    return m


def postprocess_core(scpt, Metab, b_loc=B_LOC):
    """scpt [128, 800] f32: per g2-half, V rows hold scores, P rows hold the
    MetaW projections. Softmax + e-weighted combine on host."""
    sc = np.empty((b_loc, S), np.float32)
    P = np.empty((b_loc, 4, S), np.float32)
    for g2 in range(2):
        blk = scpt[:, 400 * g2:400 * (g2 + 1)]
        for wl in range(16):
            vrow = 32 * (wl % 4) + 16 + wl // 4
            prow = 32 * ((wl + 2) % 4) + 4 * (wl // 4)
            for par in range(2):
                b = 32 * g2 + 2 * wl + par
                sc[b] = blk[vrow, 200 * par:200 * (par + 1)]
                P[b] = blk[prow:prow + 4, 200 * par:200 * (par + 1)]
    sc64 = sc.astype(np.float64)
    e = np.exp(sc64 - sc64.max(axis=1, keepdims=True))
    alpha = (e / e.sum(axis=1, keepdims=True)).astype(np.float32)
    return np.einsum('bs,bms->bm', alpha, P) + Metab.reshape(1, 4)


_cache = {}


def _get_nc():
    if "nc" not in _cache:
        _cache["nc"] = build_nc()
    return _cache["nc"]


def kernel(all_memory, last_memory, U, W, V, MetaW, Metab):
    all_memory = np.asarray(all_memory, dtype=np.float32)
    last_memory = np.asarray(last_memory, dtype=np.float32)
    U = np.asarray(U, dtype=np.float32)
    W = np.asarray(W, dtype=np.float32)
    V = np.asarray(V, dtype=np.float32)
    MetaW = np.asarray(MetaW, dtype=np.float32)
    Metab = np.asarray(Metab, dtype=np.float32)
    nc = _get_nc()
    in_maps = []
    for c in range(N_CORES):
        sl = slice(c * B_LOC, (c + 1) * B_LOC)
        in_maps.append(prep_core_inputs(
            all_memory[sl], last_memory[sl], U, W, V, MetaW))
    res = run_bass_kernel_spmd(nc, in_maps, core_ids=list(range(N_CORES)))
    outs = [postprocess_core(res.results[c]["SCPT"], Metab)
            for c in range(N_CORES)]
    return np.concatenate(outs, axis=0).astype(np.float32)
